# revision 12
# baseline (speedup 1.0000x reference)
"""CORAL focal multi-task loss on 8 Trainium2 NeuronCores.

Data-parallel over the 2M-row batch: 8 shards of 250k rows, padded to
128*2033 rows/core. Single-ACT-pass design:

Math. For one element with logit x, ordinal bit b = (t > c), kl weight w:
  loss_elem = w * (0.75 - 0.5 b) * Fc((1-2b) * x),
  Fc(z) = sigmoid(z)^2 * softplus(z)
since  -log(sigmoid(z)) = softplus(-z)  and  1 - sigmoid(z) = sigmoid(-z).

Host encode (layout + affine only; all transcendental math on device):
  z'' = x          if b=0        (z'' in [-5.5, 5.5])
      = -x - 16    if b=1        (z'' in [-21.5, -10.5], disjoint)
  quantized to int8 v = clip(rint(z''/S), -128, 127), S = 11/64; padding
  slots get v=-128 (-> -22.0 -> 0 in the table). Rows are sorted by kl_t
  per core so every SBUF partition holds one class; class segments are
  padded to multiples of RPP rows.

Device (per core):
  DMA int8 tiles [128, W] (2.6MB total) ->
  ScalarE: g = H(S*v) in ONE activation pass (free scale affine), where H
  is a custom table written into the `exp` slot of every exp-bearing set:
      H(u) = 0.75*Fc(u)      u > -10        (b=0)
           = 0.25*Fc(u+16)   -21.9 < u <= -10   (b=1)
           = 0               u <= -21.9     (padding)
    exp's stock bucket geometry has <=0.25-wide buckets over the whole
    range, so cubic Taylor per bucket gives ~5e-6 abs accuracy.
  PE: w^T @ g chunks (lhsT = per-partition class weight) accumulate into
  3 PSUM [1,512] tiles. The free dim is task-major ([4*RPP kl | 3*RPP
  jsnm | 3*RPP jsnl] per partition), so each task's sum finalizes as
  early as possible and its PSUM copy + output DMA overlap later tiles.

Engine budget per core (steady state): ACT ~18us (bottleneck, 1 elem/
lane/cycle @1.2GHz), DMA 2.6MB ~8us, PE ~8.6us, DVE ~0.
"""

import json
import os
import shutil
import numpy as np

import concourse.bacc as bacc
import concourse.mybir as mybir
import concourse.tile as tile
from concourse.bass_utils import run_bass_kernel_spmd

AluOp = mybir.AluOpType
ActFn = mybir.ActivationFunctionType
dt = mybir.dt

N = 2_000_000
NCORES = 8
NCORE = N // NCORES            # 250_000 rows per core
RPP = 2033                     # rows per partition; 123*RPP >= 250000 so 5
                               # class segments each padded to a multiple of
                               # RPP always fit in 128 partitions
NPAD = 128 * RPP
NCOLS = 10                     # 4 kl + 3 jsnm + 3 jsnl
TASKS = [(0, 4), (4, 3), (7, 3)]  # (column offset, n columns)
FTOT = NCOLS * RPP             # free-dim stream length per partition
TASK_BOUNDS = [0, 4 * RPP, 7 * RPP, 10 * RPP]
# tile windows over the free-dim stream: small first (DMA ramp) and small
# last (drain)
TILE_W = [0, 1024, 5120, 9216, 13312, 17408, 20074, FTOT]
OFF = 16.0                     # b=1 branch offset
SCALE = 11.0 / 64.0            # int8 dequant scale (activation affine)
ZCUT = -21.9                   # below this the table is exactly 0 (padding)
TABLE_VERSION = "exp_coral_v3"

EXP_SETS = ["exp_and_others", "exp_and_friends", "natural_log_exp_and_others"]


def _actroot_dir():
    base = os.path.dirname(os.path.abspath(__file__))
    cand = os.path.join(base, "actroot")
    try:
        os.makedirs(cand, exist_ok=True)
        probe = os.path.join(cand, ".w")
        open(probe, "w").write("x")
        os.remove(probe)
        return cand
    except OSError:
        import tempfile
        return os.path.join(tempfile.gettempdir(), "coral_actroot")


ACTROOT = _actroot_dir()

_CACHED = {}


# ---------------------------------------------------------------------------
# Custom activation table
# ---------------------------------------------------------------------------

def _fc(z):
    z = np.asarray(z, dtype=np.float64)
    u = np.where(z >= 0, 1.0 / (1.0 + np.exp(-np.abs(z))),
                 np.exp(-np.abs(z)) / (1.0 + np.exp(-np.abs(z))))
    sp = np.logaddexp(0.0, z)
    return u * u * sp


def _fc1(z):
    z = np.asarray(z, dtype=np.float64)
    u = np.where(z >= 0, 1.0 / (1.0 + np.exp(-np.abs(z))),
                 np.exp(-np.abs(z)) / (1.0 + np.exp(-np.abs(z))))
    sp = np.logaddexp(0.0, z)
    return u * u * (2.0 * (1.0 - u) * sp + u)


def _h_coeffs(x0):
    """Taylor coefficients [H, H', H''/2, H'''/6] of H at each x0 (f64)."""
    x0 = np.asarray(x0, dtype=np.float64)
    shifted = x0 <= -10.0
    z = np.where(shifted, x0 + OFF, x0)
    amp = np.where(shifted, 0.25, 0.75)
    h = 1e-3
    d0 = amp * _fc(z)
    d1 = amp * _fc1(z)
    d2 = amp * (_fc1(z + h) - _fc1(z - h)) / (2.0 * h) / 2.0
    d3 = amp * (_fc1(z + h) - 2.0 * _fc1(z) + _fc1(z - h)) / (h * h) / 6.0
    zero = x0 <= ZCUT
    for d in (d0, d1, d2, d3):
        d[zero] = 0.0
    # int8 dequant bias cancellation: the encoder rounds z''/S to the cell
    # center, so decode the conditional cell mean E[H(u+eps)], eps uniform
    # on +-S/2:  H + H''*S^2/24  (H'' = 2*d2)
    d0 = d0 + d2 * (SCALE * SCALE / 12.0)
    return d0, d1, d2, d3


def _ensure_actroot():
    """Build ACTROOT (idempotent) from the stock pwp_bin_trainium dir."""
    marker = os.path.join(ACTROOT, ".{}".format(TABLE_VERSION))
    if os.path.exists(marker):
        return
    from neuronxcc.driver.Job import Job
    from neuronxcc.driver.jobs.support.FindActInfo import findActInfoFile

    src = os.path.dirname(findActInfoFile(Job.getPackageDir(), "gen3"))
    os.makedirs(ACTROOT, exist_ok=True)
    for f in os.listdir(src):
        if f.startswith("."):
            continue
        dst = os.path.join(ACTROOT, f)
        if os.path.exists(dst):
            os.chmod(dst, 0o644)
        shutil.copy(os.path.join(src, f), dst)
    for f in os.listdir(ACTROOT):
        if f.startswith("."):
            os.remove(os.path.join(ACTROOT, f))

    h0 = _h_coeffs(np.array([0.0]))
    t0 = [float(c[0]) for c in h0]      # Taylor of H at 0 (b=0 branch)
    fz_bits = int(np.float32(t0[0]).view(np.uint32))

    for setname in EXP_SETS:
        pj_path = os.path.join(ACTROOT, setname + ".json")
        pj = json.load(open(pj_path))
        starts = pj["func_to_bkt_start_idx"]
        s = starts["exp"]
        nexts = [v for v in starts.values() if v > s]
        end = min(nexts) if nexts else pj["bkt_entry_cnt"]

        ent = [e for e in pj["profile_meta_data"]
               if e["func_name"].startswith("exp")][0]
        sp_small_pos = ent["pos_small_signal_pwl_control"]
        sp_small_neg = ent["neg_small_signal_pwl_control"]
        sp_large_pos = ent["pos_large_signal_pwl_control"]
        sp_large_neg = ent["neg_large_signal_pwl_control"]
        specials = {sp_small_pos, sp_small_neg, sp_large_pos, sp_large_neg}

        bkt_path = os.path.join(ACTROOT, pj["bkt_bin"])
        e = np.frombuffer(open(bkt_path, "rb").read(),
                          dtype=np.float32).reshape(-1, 8).copy()
        dense = np.array([i for i in range(s, end) if i not in specials])
        x0 = e[dense, 4].astype(np.float64)
        d0, d1, d2, d3 = _h_coeffs(x0)
        e[dense, 0] = d0
        e[dense, 1] = d1
        e[dense, 2] = d2
        e[dense, 3] = d3
        for i in (sp_small_pos, sp_small_neg):
            e[i, 0:4] = t0
            e[i, 4] = 0.0
        for i in (sp_large_pos, sp_large_neg):
            e[i, 0:5] = 0.0
        os.chmod(bkt_path, 0o644)
        open(bkt_path, "wb").write(e.tobytes())

        ent["fzero_result"] = fz_bits
        ent["fpinf_result"] = 0
        ent["fninf_result"] = 0
        os.chmod(pj_path, 0o644)
        json.dump(pj, open(pj_path, "w"), indent=1)

    open(marker, "w").write("ok")


def _segments():
    """Per tile: list of (sbuf_off, task, ln) matmul chunks, 512-aligned to
    each task's start so the first chunk of every task is full width."""
    out = []
    for j in range(len(TILE_W) - 1):
        a, b = TILE_W[j], TILE_W[j + 1]
        chunks = []
        for t in range(3):
            ta, tb = TASK_BOUNDS[t], TASK_BOUNDS[t + 1]
            lo, hi = max(a, ta), min(b, tb)
            off = lo
            while off < hi:
                ln = min(512 - ((off - ta) % 512), hi - off)
                chunks.append((off - a, t, ln))
                off += ln
        out.append(chunks)
    return out


def _build_nc(rep=1):
    nc = bacc.Bacc("TRN2", num_devices=NCORES)

    xb = nc.dram_tensor("xb", [NPAD * NCOLS], dt.int8, kind="ExternalInput")
    wv = nc.dram_tensor("wv", [128], dt.float16, kind="ExternalInput")
    pos = [nc.dram_tensor(f"po{t}", [1, 512], dt.float32,
                          kind="ExternalOutput") for t in range(3)]

    seg = _segments()
    total_mm = [sum(1 for chunks in seg for (_, tt, _) in chunks if tt == t)
                for t in range(3)]

    with tile.TileContext(nc) as tc:
        with (
            tc.tile_pool(name="singles", bufs=1) as singles,
            tc.tile_pool(name="io", bufs=3) as io,
            tc.tile_pool(name="wk", bufs=3) as wk,
            tc.tile_pool(name="ps", bufs=1, space="PSUM") as psp,
        ):
            wt = singles.tile([128, 1], dt.float16)

            pss = [psp.tile([1, 512], dt.float32, tag=f"ps{t}", name=f"ps{t}")
                   for t in range(3)]
            outts = [singles.tile([1, 512], dt.float32, name=f"outt{t}")
                     for t in range(3)]
            started = [False] * 3
            done_mm = [0] * 3

            import contextlib
            loop_ctx = (tc.For_i(0, rep, 1, hint_engines=(
                mybir.EngineType.Activation, mybir.EngineType.SP,
                mybir.EngineType.PE, mybir.EngineType.DVE)) if rep > 1
                else contextlib.nullcontext())
            with loop_ctx:
                for j in range(len(TILE_W) - 1):
                    a, b = TILE_W[j], TILE_W[j + 1]
                    F = b - a
                    xt = io.tile([128, F], dt.int8, tag="xt")
                    nc.sync.dma_start(
                        out=xt[:],
                        in_=xb[128 * a:128 * b].rearrange(
                            "(p f) -> p f", p=128))
                    if j == 0:
                        # weight vector: queued right behind tile 0's data
                        # so it doesn't delay the first activation
                        nc.sync.dma_start(
                            out=wt[:],
                            in_=wv[:].rearrange("(p f) -> p f", p=128))

                    at = wk.tile([128, F], dt.float16, tag="at")
                    nc.scalar.activation(at[:], xt[:], ActFn.Exp, scale=SCALE)

                    for (off, t, ln) in seg[j]:
                        first = not started[t]
                        done_mm[t] += 1
                        last = done_mm[t] == total_mm[t]
                        nc.tensor.matmul(
                            pss[t][0:1, 0:ln], wt[:, 0:1],
                            at[:, off:off + ln],
                            start=first, stop=last)
                        started[t] = True
                        if last and rep == 1:
                            nc.vector.tensor_copy(outts[t][:, :],
                                                  pss[t][0:1, :])
                            nc.sync.dma_start(out=pos[t][:, :],
                                              in_=outts[t][:])
            if rep > 1:
                for t in range(3):
                    nc.vector.tensor_copy(outts[t][:, :], pss[t][0:1, :])
                    nc.sync.dma_start(out=pos[t][:, :], in_=outts[t][:])

    nc.compile()
    return nc


def _prep_core(core, kl_logits, jsnm_logits, jsnl_logits, kl_t, jsnm_t,
               jsnl_t, class_weights):
    """Per-core (xb, wv): rows sorted by kl_t into single-class partitions,
    z''-encoded int8 columns, task-major free dim, tiled by TILE_W."""
    lo, hi = core * NCORE, (core + 1) * NCORE
    kl = kl_t[lo:hi]
    order = np.argsort(kl, kind="stable")
    counts = np.bincount(kl, minlength=5)[:5]
    m = ((counts + RPP - 1) // RPP) * RPP
    seg = np.concatenate([[0], np.cumsum(m)])

    Z = np.full((NCOLS, NPAD), -1000.0, dtype=np.float32)
    logits = (kl_logits[lo:hi], jsnm_logits[lo:hi], jsnl_logits[lo:hi])
    targs = (kl, jsnm_t[lo:hi], jsnl_t[lo:hi])
    # destination slots of the real rows (class segments, padded to RPP);
    # `order` (stable argsort by kl) is already grouped by class, in class
    # order, so slot i holds row order[i]
    slots = np.concatenate([
        np.arange(seg[c], seg[c] + counts[c]) for c in range(5)])
    for tsk, (coff, C) in enumerate(TASKS):
        xg = logits[tsk][order]          # [NCORE, C] in destination order
        tg = targs[tsk][order]
        for c in range(C):
            b = tg > c
            Z[coff + c, slots] = np.where(b, -xg[:, c] - OFF, xg[:, c])

    V = np.clip(np.rint(Z / SCALE), -128, 127).astype(np.int8)
    # [128, 10, RPP] -> per-partition task-major stream [128, 10*RPP]
    flat = V.reshape(NCOLS, 128, RPP).transpose(1, 0, 2).reshape(128, FTOT)
    xbv = np.concatenate([flat[:, TILE_W[j]:TILE_W[j + 1]].ravel()
                          for j in range(len(TILE_W) - 1)])

    # per-partition class weight (0 for all-padding partitions)
    w = np.zeros(128, dtype=np.float32)
    for c in range(5):
        p0, p1 = seg[c] // RPP, (seg[c] + m[c]) // RPP
        w[p0:p1] = class_weights[c]
    return xbv, w.astype(np.float16)


def kernel(kl_logits, jsnm_logits, jsnl_logits, class_weights, kl_t,
           jsnm_t, jsnl_t):
    kl_logits = np.asarray(kl_logits, dtype=np.float32)
    jsnm_logits = np.asarray(jsnm_logits, dtype=np.float32)
    jsnl_logits = np.asarray(jsnl_logits, dtype=np.float32)
    class_weights = np.asarray(class_weights, dtype=np.float32)
    kl_t = np.asarray(kl_t).astype(np.int32)
    jsnm_t = np.asarray(jsnm_t).astype(np.int32)
    jsnl_t = np.asarray(jsnl_t).astype(np.int32)

    _ensure_actroot()
    os.environ["BASS_ACT_ROOT_JSON_PATH"] = os.path.join(
        ACTROOT, "act_info.json")

    if "nc" not in _CACHED:
        _CACHED["nc"] = _build_nc()
    nc = _CACHED["nc"]

    in_maps = []
    for core in range(NCORES):
        xbv, wvv = _prep_core(core, kl_logits, jsnm_logits, jsnl_logits,
                              kl_t, jsnm_t, jsnl_t, class_weights)
        in_maps.append({"xb": xbv, "wv": wvv})

    res = run_bass_kernel_spmd(nc, in_maps, core_ids=list(range(NCORES)),
                               trace=False)

    S = np.zeros(3, dtype=np.float64)
    for core in range(NCORES):
        for t in range(3):
            S[t] += res.results[core][f"po{t}"].astype(np.float64).sum()

    l_kl = S[0] / (N * 4)
    l_m = S[1] / (N * 3)
    l_l = S[2] / (N * 3)
    total = (l_kl + l_m + l_l) / 3.0
    return (np.float32(total), np.float32(l_kl), np.float32(l_m),
            np.float32(l_l))
